# revision 3
# baseline (speedup 1.0000x reference)
"""nn_ContrastiveLoss Trainium2 kernel (8 NeuronCores, data-parallel over batch).

Contract: kernel(embeddings=[64,1024,128] f32, labels=[64,1024] int64) -> f32 scalar.

Sharding: batch dim B=64 split as 8 samples per core. Each core computes, for
each of its samples, the hinge-loss numerator S_b = sum_{i in pos, j in neg}
relu(cos_sim(i,j) - 0.15), the per-sample loss S_b*valid/max(n_neg,1) and the
count n_pos*valid, entirely on device. Host sums the 8x[2,8] partials and does
the final division (the "all-reduce" of the sharding hint).

Device pipeline per sample (engines balanced to overlap):
  - DMA embeddings as [128p, 8t, 128d] (row n = t*128+p)
  - row norms: ACT square -> DVE reduce -> sqrt/reciprocal (fp32)
  - PE transposes E_t^T via matmul with identity (lhsT side, raw) and with
    diag(negmask*rinv) (rhs side: fused column masking + normalization)
  - PSUM->SBUF copies cast to bf16 (ACT/DVE split)
  - PE sim matmuls in bf16 (fp32 PSUM accumulate)
  - hinge + row reduction fused: ACT relu(scale=rinv_i, bias=-0.15, accum_out)
    for even row-tiles; DVE max(sim, 0.15*r_i) + add-reduce (equivalent via
    max-trick) for odd row-tiles
  - pos-mask dot + cross-partition all-reduce (GpSimd) -> [2, 8] output
"""

import sys

if "/opt/trn_rl_repo" not in sys.path:
    sys.path.insert(0, "/opt/trn_rl_repo")

from contextlib import ExitStack

import numpy as np

import concourse.bass as bass
import concourse.bacc as bacc
import concourse.mybir as mybir
import concourse.tile as tile
from concourse import bass_isa, bass_utils

F32 = mybir.dt.float32
BF16 = mybir.dt.bfloat16
AF = mybir.ActivationFunctionType
ALU = mybir.AluOpType

P = 128      # SBUF partitions
D = 128      # embedding dim
N = 1024     # rows per sample
T = N // P   # row-chunks per sample
B = 64       # full batch
NCORES = 8
BPC = B // NCORES
THRESH = 0.5 - 0.35   # margin threshold 0.15
EPS = 1e-6


def _kernel_body(ctx, tc, emb_ap, lab_ap, out_ap, bpc):
    nc = tc.nc

    const_pool = ctx.enter_context(tc.tile_pool(name="const", bufs=1))
    epool = ctx.enter_context(tc.tile_pool(name="epool", bufs=2))
    etpool = ctx.enter_context(tc.tile_pool(name="etpool", bufs=2))
    diagpool = ctx.enter_context(tc.tile_pool(name="diagpool", bufs=2))
    scrpool_a = ctx.enter_context(tc.tile_pool(name="scra", bufs=2))
    scrpool_d = ctx.enter_context(tc.tile_pool(name="scrd", bufs=2))
    small = ctx.enter_context(tc.tile_pool(name="small", bufs=2))
    acc_pool = ctx.enter_context(tc.tile_pool(name="acc", bufs=1))
    pspool = ctx.enter_context(tc.tile_pool(name="ps", bufs=4, space="PSUM"))

    ones_col = const_pool.tile([P, 1], F32)
    nc.gpsimd.memset(ones_col[:], 1.0)
    neg_thr = const_pool.tile([P, 1], F32)
    nc.gpsimd.memset(neg_thr[:], -THRESH)
    ident = const_pool.tile([P, P], F32)
    nc.gpsimd.affine_select(
        ident[:], ones_col[:].broadcast_to([P, P]),
        pattern=[[-1, P]], compare_op=ALU.is_equal, fill=0.0,
        base=0, channel_multiplier=1,
    )

    posm = const_pool.tile([P, bpc, T], F32)
    nc.sync.dma_start(posm[:], lab_ap.rearrange("b (t p) -> p b t", p=P))
    negm = const_pool.tile([P, bpc, T], F32)
    nc.vector.tensor_scalar(negm[:], posm[:], -1.0, 1.0, ALU.mult, ALU.add)

    s_col = acc_pool.tile([P, bpc], F32)
    np_col = acc_pool.tile([P, bpc], F32)
    nc.vector.memzero(s_col[:])
    nc.vector.memzero(np_col[:])

    for b in range(bpc):
        e_nat = epool.tile([P, T, D], F32, tag="e_nat")
        nc.sync.dma_start(e_nat[:], emb_ap[b].rearrange("(t p) d -> p t d", p=P))

        esq = epool.tile([P, T, D], F32, tag="esq")
        nc.scalar.activation(esq[:], e_nat[:], AF.Square)
        nsq = small.tile([P, T], F32, tag="nsq")
        nc.vector.tensor_reduce(nsq[:], esq[:], axis=mybir.AxisListType.X,
                                op=ALU.add)
        r_ = small.tile([P, T], F32, tag="r_")
        nc.scalar.activation(r_[:], nsq[:], AF.Sqrt)
        rc = small.tile([P, T], F32, tag="rc")
        nc.vector.tensor_scalar_max(rc[:], r_[:], EPS)
        rinv = small.tile([P, T], F32, tag="rinv")
        nc.vector.reciprocal(rinv[:], rc[:])
        thr = small.tile([P, T], F32, tag="thr")
        nc.scalar.mul(thr[:], rc[:], THRESH)
        comb = small.tile([P, T], F32, tag="comb")
        nc.vector.tensor_mul(comb[:], negm[:, b, :], rinv[:])

        diagall = diagpool.tile([P, T, D], F32, tag="diag")
        nc.gpsimd.affine_select(
            diagall[:], comb[:].unsqueeze(2).broadcast_to([P, T, D]),
            pattern=[[0, T], [-1, D]], compare_op=ALU.is_equal, fill=0.0,
            base=0, channel_multiplier=1,
        )

        traw_ps = pspool.tile([P, N], F32, tag="ps")
        tmsk_ps = pspool.tile([P, N], F32, tag="ps")
        for t in range(T):
            nc.tensor.matmul(traw_ps[:, bass.ts(t, P)], lhsT=e_nat[:, t, :],
                             rhs=ident[:], start=True, stop=True)
            nc.tensor.matmul(tmsk_ps[:, bass.ts(t, P)], lhsT=e_nat[:, t, :],
                             rhs=diagall[:, t, :], start=True, stop=True)
        et_raw = etpool.tile([P, N], BF16, tag="et_raw")
        nc.scalar.copy(et_raw[:], traw_ps[:])
        et_msk = etpool.tile([P, N], BF16, tag="et_msk")
        nc.vector.tensor_copy(et_msk[:], tmsk_ps[:])

        slot_a = small.tile([P, T // 2], F32, tag="slot_a")
        slot_d = small.tile([P, T // 2], F32, tag="slot_d")
        for mt in range(T):
            sim_ps = pspool.tile([P, N], F32, tag="ps")
            for jt in range(N // 512):
                nc.tensor.matmul(sim_ps[:, bass.ts(jt, 512)],
                                 lhsT=et_raw[:, bass.ts(mt, P)],
                                 rhs=et_msk[:, bass.ts(jt, 512)],
                                 start=True, stop=True)
            if mt % 2 == 0:
                scr = scrpool_a.tile([P, N], F32, tag="scra")
                nc.scalar.activation(
                    scr[:], sim_ps[:], AF.Relu,
                    bias=neg_thr[:], scale=rinv[:, mt:mt + 1],
                    accum_out=slot_a[:, mt // 2:mt // 2 + 1])
            else:
                # out = max(sim, thr_i); accum = sum_j(out)
                # (sum_j max(sim,thr) - N*thr == sum_j relu(sim - thr))
                scr = scrpool_d.tile([P, N], F32, tag="scrd")
                nc.vector.tensor_scalar(
                    scr[:], sim_ps[:], thr[:, mt:mt + 1], None,
                    ALU.max, ALU.add,
                    accum_out=slot_d[:, mt // 2:mt // 2 + 1])

        corr = small.tile([P, T // 2], F32, tag="corr")
        nc.vector.tensor_scalar_mul(corr[:], thr[:, 1::2], float(N))
        vd = small.tile([P, T // 2], F32, tag="vd")
        nc.vector.tensor_sub(vd[:], slot_d[:], corr[:])
        vd2 = small.tile([P, T // 2], F32, tag="vd2")
        nc.vector.tensor_mul(vd2[:], vd[:], rinv[:, 1::2])

        pa = small.tile([P, T // 2], F32, tag="pa")
        nc.vector.tensor_mul(pa[:], slot_a[:], posm[:, b, 0::2])
        sa = small.tile([P, 1], F32, tag="sa")
        nc.vector.tensor_reduce(sa[:], pa[:], axis=mybir.AxisListType.X,
                                op=ALU.add)
        pd = small.tile([P, T // 2], F32, tag="pd")
        nc.vector.tensor_mul(pd[:], vd2[:], posm[:, b, 1::2])
        sd = small.tile([P, 1], F32, tag="sd")
        nc.vector.tensor_reduce(sd[:], pd[:], axis=mybir.AxisListType.X,
                                op=ALU.add)
        nc.vector.tensor_add(s_col[:, b:b + 1], sa[:], sd[:])

        nc.vector.tensor_reduce(np_col[:, b:b + 1], posm[:, b, :],
                                axis=mybir.AxisListType.X, op=ALU.add)

    sr = acc_pool.tile([P, bpc], F32)
    nc.gpsimd.partition_all_reduce(sr[:], s_col[:], channels=P,
                                   reduce_op=bass_isa.ReduceOp.add)
    npr = acc_pool.tile([P, bpc], F32)
    nc.gpsimd.partition_all_reduce(npr[:], np_col[:], channels=P,
                                   reduce_op=bass_isa.ReduceOp.add)

    nnr = small.tile([P, bpc], F32, tag="nnr")
    nc.vector.tensor_scalar(nnr[:], npr[:], -1.0, float(N), ALU.mult, ALU.add)
    vp = small.tile([P, bpc], F32, tag="vp")
    nc.vector.tensor_scalar(vp[:], npr[:], 0.5, None, ALU.is_gt)
    vn = small.tile([P, bpc], F32, tag="vn")
    nc.vector.tensor_scalar(vn[:], nnr[:], 0.5, None, ALU.is_gt)
    valid = small.tile([P, bpc], F32, tag="valid")
    nc.vector.tensor_mul(valid[:], vp[:], vn[:])
    nnc = small.tile([P, bpc], F32, tag="nnc")
    nc.vector.tensor_scalar_max(nnc[:], nnr[:], 1.0)
    nninv = small.tile([P, bpc], F32, tag="nninv")
    nc.vector.reciprocal(nninv[:], nnc[:])

    sv = small.tile([P, bpc], F32, tag="sv")
    nc.vector.tensor_mul(sv[:], sr[:], valid[:])
    lossv = small.tile([P, bpc], F32, tag="lossv")
    nc.vector.tensor_mul(lossv[:], sv[:], nninv[:])
    cntv = small.tile([P, bpc], F32, tag="cntv")
    nc.vector.tensor_mul(cntv[:], npr[:], valid[:])

    nc.sync.dma_start(out_ap[0:1, :], lossv[0:1, :])
    nc.sync.dma_start(out_ap[1:2, :], cntv[0:1, :])


_NC_CACHE = {}


def _build():
    key = (BPC, NCORES)
    if key in _NC_CACHE:
        return _NC_CACHE[key]
    nc = bacc.Bacc("TRN2", target_bir_lowering=False, debug=False,
                   num_devices=NCORES)
    emb = nc.dram_tensor("emb", [BPC, N, D], F32, kind="ExternalInput")
    lab = nc.dram_tensor("lab", [BPC, N], F32, kind="ExternalInput")
    out = nc.dram_tensor("out", [2, BPC], F32, kind="ExternalOutput")
    with tile.TileContext(nc) as tc:
        with ExitStack() as ctx:
            _kernel_body(ctx, tc, emb.ap(), lab.ap(), out.ap(), BPC)
    nc.compile()
    _NC_CACHE[key] = nc
    return nc


def kernel(embeddings: np.ndarray, labels: np.ndarray,
           _want_results=False, _trace=False) -> np.ndarray:
    emb = np.ascontiguousarray(embeddings, dtype=np.float32)
    lab_f = np.ascontiguousarray(labels.astype(np.float32))
    assert emb.shape == (B, N, D) and lab_f.shape == (B, N)

    nc = _build()
    in_maps = [
        {"emb": emb[c * BPC:(c + 1) * BPC], "lab": lab_f[c * BPC:(c + 1) * BPC]}
        for c in range(NCORES)
    ]
    res = bass_utils.run_bass_kernel_spmd(nc, in_maps,
                                          core_ids=list(range(NCORES)),
                                          trace=_trace)
    loss_sum = 0.0
    count = 0.0
    for c in range(NCORES):
        o = res.results[c]["out"]
        loss_sum += float(o[0].sum())
        count += float(o[1].sum())
    ans = np.float32(loss_sum) / np.float32(max(count, 1.0))
    if _want_results:
        return np.float32(ans), res
    return np.float32(ans)


# revision 4
# speedup vs baseline: 1.2191x; 1.2191x over previous
"""nn_ContrastiveLoss Trainium2 kernel (8 NeuronCores, data-parallel over batch).

Contract: kernel(embeddings=[64,1024,128] f32, labels=[64,1024] int64) -> f32 scalar.

Sharding: batch dim B=64 split as 8 samples per core. Host-side sharding also
packs each sample's rows by label (positives first, then negatives, each
zero-padded to a 128-row multiple) so the device computes the pos x neg hinge
matrix densely instead of the full 1024x1024 with masks. Each core returns
[2, bpc] per-sample (loss, count) partials; host sums across cores and divides
(the all-reduce + final division of the sharding hint).

Device pipeline per sample:
  - DMA packed rows as [128p, TT t, 128d] f32 (row k = t*128 + p)
  - row norms: ACT square -> DVE reduce -> ACT sqrt -> DVE reciprocal
  - cast rows to bf16 (GpSimd copy); diag(rinv) tiles via GpSimd affine_select
  - PE transposes: E_t^T @ diag(rinv_t) -> normalized transposed chunks
    (zero pad rows have rinv=1/eps but x=0, so their columns stay 0)
  - PSUM->SBUF copies cast to bf16 (ACT for pos block, DVE for neg block)
  - PE sim matmuls bf16: sim = et_pos_chunk^T . et_neg -> PSUM fp32
  - fused hinge+reduce, scale-free since sim is fully normalized:
      ACT row-tiles: relu(sim - 0.15) with accum_out
      DVE row-tiles: max(sim, 0.15) summed, then -PADN*0.15 correction
  - per-sample counts from nsq>0 (pad rows have zero norm)
  - GpSimd cross-partition all-reduce -> [2, bpc] output
"""

import sys

if "/opt/trn_rl_repo" not in sys.path:
    sys.path.insert(0, "/opt/trn_rl_repo")

from contextlib import ExitStack

import numpy as np

import concourse.bass as bass
import concourse.bacc as bacc
import concourse.mybir as mybir
import concourse.tile as tile
from concourse import bass_isa, bass_utils

F32 = mybir.dt.float32
BF16 = mybir.dt.bfloat16
AF = mybir.ActivationFunctionType
ALU = mybir.AluOpType

P = 128      # SBUF partitions
D = 128      # embedding dim
N = 1024     # rows per sample
B = 64       # full batch
NCORES = 8
BPC = B // NCORES
THRESH = 0.5 - 0.35   # margin threshold 0.15
EPS = 1e-6


def _kernel_body(ctx, tc, emb_ap, out_ap, bpc, padp, padn):
    nc = tc.nc
    tp, tn = padp // P, padn // P
    tt = tp + tn

    const_pool = ctx.enter_context(tc.tile_pool(name="const", bufs=1))
    epool = ctx.enter_context(tc.tile_pool(name="epool", bufs=2))
    etpool = ctx.enter_context(tc.tile_pool(name="etpool", bufs=2))
    diagpool = ctx.enter_context(tc.tile_pool(name="diagpool", bufs=2))
    scrpool_a = ctx.enter_context(tc.tile_pool(name="scra", bufs=2))
    scrpool_d = ctx.enter_context(tc.tile_pool(name="scrd", bufs=2))
    small = ctx.enter_context(tc.tile_pool(name="small", bufs=2))
    acc_pool = ctx.enter_context(tc.tile_pool(name="acc", bufs=1))
    tr_psum = ctx.enter_context(tc.tile_pool(name="trps", bufs=2, space="PSUM"))
    sim_psum = ctx.enter_context(tc.tile_pool(name="simps", bufs=2, space="PSUM"))

    neg_thr = const_pool.tile([P, 1], F32)
    nc.gpsimd.memset(neg_thr[:], -THRESH)

    s_col = acc_pool.tile([P, bpc], F32)
    np_col = acc_pool.tile([P, bpc], F32)
    nn_col = acc_pool.tile([P, bpc], F32)

    for b in range(bpc):
        e_nat = epool.tile([P, tt, D], F32, tag="e_nat")
        nc.sync.dma_start(e_nat[:], emb_ap[b].rearrange("(t p) d -> p t d", p=P))

        esq = epool.tile([P, tt, D], F32, tag="esq")
        nc.scalar.activation(esq[:], e_nat[:], AF.Square)
        nsq = small.tile([P, tt], F32, tag="nsq")
        nc.vector.tensor_reduce(nsq[:], esq[:], axis=mybir.AxisListType.X,
                                op=ALU.add)
        r_ = small.tile([P, tt], F32, tag="r_")
        nc.scalar.activation(r_[:], nsq[:], AF.Sqrt)
        rc = small.tile([P, tt], F32, tag="rc")
        nc.vector.tensor_scalar_max(rc[:], r_[:], EPS)
        rinv = small.tile([P, tt], F32, tag="rinv")
        nc.vector.reciprocal(rinv[:], rc[:])
        rinv_bf = small.tile([P, tt], BF16, tag="rinv_bf")
        nc.vector.tensor_copy(rinv_bf[:], rinv[:])

        e_bf = epool.tile([P, tt, D], BF16, tag="e_bf")
        nc.gpsimd.tensor_copy(e_bf[:], e_nat[:])
        diagall = diagpool.tile([P, tt, D], BF16, tag="diag")
        nc.gpsimd.affine_select(
            diagall[:], rinv_bf[:].unsqueeze(2).broadcast_to([P, tt, D]),
            pattern=[[0, tt], [-1, D]], compare_op=ALU.is_equal, fill=0.0,
            base=0, channel_multiplier=1,
        )

        # normalized transposes: chunk^T @ diag(rinv_chunk) -> fp32 PSUM
        ps_p = tr_psum.tile([P, padp], F32, tag="trps")
        ps_n = tr_psum.tile([P, padn], F32, tag="trps")
        for t in range(tp):
            nc.tensor.matmul(ps_p[:, bass.ts(t, P)], lhsT=e_bf[:, t, :],
                             rhs=diagall[:, t, :], start=True, stop=True)
        for t in range(tn):
            nc.tensor.matmul(ps_n[:, bass.ts(t, P)], lhsT=e_bf[:, tp + t, :],
                             rhs=diagall[:, tp + t, :], start=True, stop=True)
        et_p = etpool.tile([P, padp], BF16, tag="et_p")
        nc.scalar.copy(et_p[:], ps_p[:])
        et_n = etpool.tile([P, padn], BF16, tag="et_n")
        nc.vector.tensor_copy(et_n[:], ps_n[:])

        # sim matmuls + fused scale-free hinge reduction
        slot = small.tile([P, tp], F32, tag="slot")
        for mt in range(tp):
            sim_ps = sim_psum.tile([P, padn], F32, tag="simps")
            for j0 in range(0, padn, 512):
                jw = min(512, padn - j0)
                nc.tensor.matmul(sim_ps[:, j0:j0 + jw],
                                 lhsT=et_p[:, bass.ts(mt, P)],
                                 rhs=et_n[:, j0:j0 + jw],
                                 start=True, stop=True)
            if mt % 2 == 0:
                scr = scrpool_a.tile([P, padn], F32, tag="scra")
                nc.scalar.activation(scr[:], sim_ps[:], AF.Relu,
                                     bias=neg_thr[:],
                                     accum_out=slot[:, mt:mt + 1])
            else:
                scr = scrpool_d.tile([P, padn], F32, tag="scrd")
                nc.vector.tensor_scalar(scr[:], sim_ps[:], THRESH, None,
                                        ALU.max, ALU.add,
                                        accum_out=slot[:, mt:mt + 1])

        # DVE slots counted max(sim, t): subtract padn*t to get relu sums
        ndve = len(range(1, tp, 2))
        if ndve:
            slot2 = small.tile([P, ndve], F32, tag="slot2")
            nc.vector.tensor_scalar_sub(slot2[:], slot[:, 1::2],
                                        float(padn) * THRESH)
            nc.vector.tensor_copy(slot[:, 1::2], slot2[:])
        nc.vector.tensor_reduce(s_col[:, b:b + 1], slot[:],
                                axis=mybir.AxisListType.X, op=ALU.add)

        # counts: real rows have nsq > 0 (pad rows are all-zero)
        live = small.tile([P, tt], F32, tag="live")
        nc.vector.tensor_scalar(live[:], nsq[:], 0.0, None, ALU.is_gt)
        nc.vector.tensor_reduce(np_col[:, b:b + 1], live[:, 0:tp],
                                axis=mybir.AxisListType.X, op=ALU.add)
        nc.vector.tensor_reduce(nn_col[:, b:b + 1], live[:, tp:tt],
                                axis=mybir.AxisListType.X, op=ALU.add)

    sr = acc_pool.tile([P, bpc], F32)
    nc.gpsimd.partition_all_reduce(sr[:], s_col[:], channels=P,
                                   reduce_op=bass_isa.ReduceOp.add)
    npr = acc_pool.tile([P, bpc], F32)
    nc.gpsimd.partition_all_reduce(npr[:], np_col[:], channels=P,
                                   reduce_op=bass_isa.ReduceOp.add)
    nnr = acc_pool.tile([P, bpc], F32)
    nc.gpsimd.partition_all_reduce(nnr[:], nn_col[:], channels=P,
                                   reduce_op=bass_isa.ReduceOp.add)

    vp = small.tile([P, bpc], F32, tag="vp")
    nc.vector.tensor_scalar(vp[:], npr[:], 0.5, None, ALU.is_gt)
    vn = small.tile([P, bpc], F32, tag="vn")
    nc.vector.tensor_scalar(vn[:], nnr[:], 0.5, None, ALU.is_gt)
    valid = small.tile([P, bpc], F32, tag="valid")
    nc.vector.tensor_mul(valid[:], vp[:], vn[:])
    nnc = small.tile([P, bpc], F32, tag="nnc")
    nc.vector.tensor_scalar_max(nnc[:], nnr[:], 1.0)
    nninv = small.tile([P, bpc], F32, tag="nninv")
    nc.vector.reciprocal(nninv[:], nnc[:])

    sv = small.tile([P, bpc], F32, tag="sv")
    nc.vector.tensor_mul(sv[:], sr[:], valid[:])
    lossv = small.tile([P, bpc], F32, tag="lossv")
    nc.vector.tensor_mul(lossv[:], sv[:], nninv[:])
    cntv = small.tile([P, bpc], F32, tag="cntv")
    nc.vector.tensor_mul(cntv[:], npr[:], valid[:])

    nc.sync.dma_start(out_ap[0:1, :], lossv[0:1, :])
    nc.sync.dma_start(out_ap[1:2, :], cntv[0:1, :])


_NC_CACHE = {}


def _build(padp, padn):
    key = (BPC, NCORES, padp, padn)
    if key in _NC_CACHE:
        return _NC_CACHE[key]
    nc = bacc.Bacc("TRN2", target_bir_lowering=False, debug=False,
                   num_devices=NCORES)
    emb = nc.dram_tensor("emb", [BPC, padp + padn, D], F32,
                         kind="ExternalInput")
    out = nc.dram_tensor("out", [2, BPC], F32, kind="ExternalOutput")
    with tile.TileContext(nc) as tc:
        with ExitStack() as ctx:
            _kernel_body(ctx, tc, emb.ap(), out.ap(), BPC, padp, padn)
    nc.compile()
    _NC_CACHE[key] = nc
    return nc


def _pack(emb, labels):
    """Per-sample label packing: pos rows, zero pad, neg rows, zero pad."""
    npos = (labels == 1).sum(axis=1)
    nneg = (labels == 0).sum(axis=1)
    padp = max(P, int(-(-npos.max() // P)) * P)
    padn = max(P, int(-(-nneg.max() // P)) * P)
    packed = np.zeros((B, padp + padn, D), np.float32)
    for b in range(B):
        pos_idx = np.nonzero(labels[b] == 1)[0]
        neg_idx = np.nonzero(labels[b] == 0)[0]
        packed[b, :len(pos_idx)] = emb[b, pos_idx]
        packed[b, padp:padp + len(neg_idx)] = emb[b, neg_idx]
    return packed, padp, padn


def kernel(embeddings: np.ndarray, labels: np.ndarray,
           _want_results=False, _trace=False) -> np.ndarray:
    emb = np.ascontiguousarray(embeddings, dtype=np.float32)
    lab = np.asarray(labels)
    assert emb.shape == (B, N, D) and lab.shape == (B, N)

    packed, padp, padn = _pack(emb, lab)
    nc = _build(padp, padn)
    in_maps = [{"emb": packed[c * BPC:(c + 1) * BPC]} for c in range(NCORES)]
    res = bass_utils.run_bass_kernel_spmd(nc, in_maps,
                                          core_ids=list(range(NCORES)),
                                          trace=_trace)
    loss_sum = 0.0
    count = 0.0
    for c in range(NCORES):
        o = res.results[c]["out"]
        loss_sum += float(o[0].sum())
        count += float(o[1].sum())
    ans = np.float32(loss_sum) / np.float32(max(count, 1.0))
    if _want_results:
        return np.float32(ans), res
    return np.float32(ans)


# revision 7
# speedup vs baseline: 1.4389x; 1.1803x over previous
"""nn_ContrastiveLoss Trainium2 kernel (8 NeuronCores, data-parallel over batch).

Contract: kernel(embeddings=[64,1024,128] f32, labels=[64,1024] int64) -> f32 scalar.

Sharding: batch dim B=64 split as 8 samples per core. Host-side sharding also
packs each sample's rows by label (positives first, then negatives, each
zero-padded to a 128-row multiple) so the device computes the pos x neg hinge
matrix densely instead of the full 1024x1024 with masks. Each core returns
[2, bpc] per-sample (loss, count) partials; host sums across cores and divides
(the all-reduce + final division of the sharding hint).

Device pipeline per sample:
  - DMA packed rows as [128p, TT t, 128d] f32 (row k = t*128 + p)
  - row norms: ACT square -> DVE reduce -> ACT sqrt -> DVE reciprocal
  - cast rows to bf16 (GpSimd copy); diag(rinv) tiles via GpSimd affine_select
  - PE transposes: E_t^T @ diag(rinv_t) -> normalized transposed chunks
    (zero pad rows have rinv=1/eps but x=0, so their columns stay 0)
  - PSUM->SBUF copies cast to bf16 (ACT for pos block, DVE for neg block)
  - PE sim matmuls bf16: sim = et_pos_chunk^T . et_neg -> PSUM fp32
  - fused hinge+reduce, scale-free since sim is fully normalized:
      ACT row-tiles: relu(sim - 0.15) with accum_out
      DVE row-tiles: max(sim, 0.15) summed, then -PADN*0.15 correction
  - per-sample counts from nsq>0 (pad rows have zero norm)
  - GpSimd cross-partition all-reduce -> [2, bpc] output
"""

import sys

if "/opt/trn_rl_repo" not in sys.path:
    sys.path.insert(0, "/opt/trn_rl_repo")

from contextlib import ExitStack

import numpy as np

import concourse.bass as bass
import concourse.bacc as bacc
import concourse.mybir as mybir
import concourse.tile as tile
from concourse import bass_isa, bass_utils

F32 = mybir.dt.float32
BF16 = mybir.dt.bfloat16
AF = mybir.ActivationFunctionType
ALU = mybir.AluOpType

P = 128      # SBUF partitions
D = 128      # embedding dim
N = 1024     # rows per sample
B = 64       # full batch
NCORES = 8
BPC = B // NCORES
THRESH = 0.5 - 0.35   # margin threshold 0.15
EPS = 1e-6


def _kernel_body(ctx, tc, emb_ap, out_ap, bpc, padp, padn):
    nc = tc.nc
    tp, tn = padp // P, padn // P
    tt = tp + tn

    const_pool = ctx.enter_context(tc.tile_pool(name="const", bufs=1))
    epool = ctx.enter_context(tc.tile_pool(name="epool", bufs=2))
    etpool = ctx.enter_context(tc.tile_pool(name="etpool", bufs=2))
    diagpool = ctx.enter_context(tc.tile_pool(name="diagpool", bufs=2))
    scrpool_a = ctx.enter_context(tc.tile_pool(name="scra", bufs=2))
    scrpool_d = ctx.enter_context(tc.tile_pool(name="scrd", bufs=2))
    small = ctx.enter_context(tc.tile_pool(name="small", bufs=2))
    acc_pool = ctx.enter_context(tc.tile_pool(name="acc", bufs=1))
    tr_psum = ctx.enter_context(tc.tile_pool(name="trps", bufs=2, space="PSUM"))
    sim_psum = ctx.enter_context(tc.tile_pool(name="simps", bufs=2, space="PSUM"))

    neg_thr = const_pool.tile([P, 1], F32)
    nc.gpsimd.memset(neg_thr[:], -THRESH)
    eps2 = const_pool.tile([P, 1], F32)
    nc.gpsimd.memset(eps2[:], EPS * EPS)

    s_col = acc_pool.tile([P, bpc], F32)
    np_col = acc_pool.tile([P, bpc], F32)
    nn_col = acc_pool.tile([P, bpc], F32)
    nsq_all = acc_pool.tile([P, bpc, tt], F32)

    for b in range(bpc):
        e_nat = epool.tile([P, tt, D], F32, tag="e_nat")
        nc.sync.dma_start(e_nat[:], emb_ap[b].rearrange("(t p) d -> p t d", p=P))

        esq = epool.tile([P, tt, D], F32, tag="esq")
        nc.scalar.activation(esq[:], e_nat[:], AF.Square)
        nsq = nsq_all[:, b, :]
        nc.vector.tensor_reduce(nsq, esq[:], axis=mybir.AxisListType.X,
                                op=ALU.add)
        # r = sqrt(nsq + eps^2) folds in the max(r, eps) clamp (pad rows)
        r_ = small.tile([P, tt], F32, tag="r_")
        nc.scalar.activation(r_[:], nsq, AF.Sqrt, bias=eps2[:])
        rinv = small.tile([P, tt], F32, tag="rinv")
        nc.vector.reciprocal(rinv[:], r_[:])

        e_bf = epool.tile([P, tt, D], BF16, tag="e_bf")
        nc.scalar.copy(e_bf[:, 0:tp, :], e_nat[:, 0:tp, :])
        nc.vector.tensor_copy(e_bf[:, tp:tt, :], e_nat[:, tp:tt, :])
        diagall = diagpool.tile([P, tt, D], BF16, tag="diag")
        nc.gpsimd.affine_select(
            diagall[:], rinv[:].unsqueeze(2).broadcast_to([P, tt, D]),
            pattern=[[0, tt], [-1, D]], compare_op=ALU.is_equal, fill=0.0,
            base=0, channel_multiplier=1,
        )

        # normalized transposes: chunk^T @ diag(rinv_chunk) -> fp32 PSUM
        ps_p = tr_psum.tile([P, padp], F32, tag="trps")
        ps_n = tr_psum.tile([P, padn], F32, tag="trps")
        for t in range(tp):
            nc.tensor.matmul(ps_p[:, bass.ts(t, P)], lhsT=e_bf[:, t, :],
                             rhs=diagall[:, t, :], start=True, stop=True)
        for t in range(tn):
            nc.tensor.matmul(ps_n[:, bass.ts(t, P)], lhsT=e_bf[:, tp + t, :],
                             rhs=diagall[:, tp + t, :], start=True, stop=True)
        et_p = etpool.tile([P, padp], BF16, tag="et_p")
        nc.scalar.copy(et_p[:], ps_p[:])
        et_n = etpool.tile([P, padn], BF16, tag="et_n")
        nc.vector.tensor_copy(et_n[:], ps_n[:])

        # sim matmuls + fused scale-free hinge reduction
        slot = small.tile([P, tp], F32, tag="slot")
        for mt in range(tp):
            sim_ps = sim_psum.tile([P, padn], F32, tag="simps")
            for j0 in range(0, padn, 512):
                jw = min(512, padn - j0)
                nc.tensor.matmul(sim_ps[:, j0:j0 + jw],
                                 lhsT=et_p[:, bass.ts(mt, P)],
                                 rhs=et_n[:, j0:j0 + jw],
                                 start=True, stop=True)
            if mt % 2 == 1:
                scr = scrpool_a.tile([P, padn], F32, tag="scra")
                nc.scalar.activation(scr[:], sim_ps[:], AF.Relu,
                                     bias=neg_thr[:],
                                     accum_out=slot[:, mt:mt + 1])
            else:
                scr = scrpool_d.tile([P, padn], F32, tag="scrd")
                nc.vector.tensor_scalar(scr[:], sim_ps[:], THRESH, None,
                                        ALU.max, ALU.add,
                                        accum_out=slot[:, mt:mt + 1])

        nc.vector.tensor_reduce(s_col[:, b:b + 1], slot[:],
                                axis=mybir.AxisListType.X, op=ALU.add)

    # DVE hinge slots (mt even) summed max(sim, t); remove ndve*padn*t once
    ndve = len(range(0, tp, 2))
    s_fix = acc_pool.tile([P, bpc], F32)
    nc.vector.tensor_scalar_sub(s_fix[:], s_col[:],
                                float(ndve) * float(padn) * THRESH)

    # counts: real rows have nsq > 0 (pad rows are all-zero)
    live = acc_pool.tile([P, bpc, tt], F32)
    nc.vector.tensor_scalar(live[:], nsq_all[:], 0.0, None, ALU.is_gt)
    nc.vector.tensor_reduce(np_col[:], live[:, :, 0:tp],
                            axis=mybir.AxisListType.X, op=ALU.add)
    nc.vector.tensor_reduce(nn_col[:], live[:, :, tp:tt],
                            axis=mybir.AxisListType.X, op=ALU.add)

    sr = acc_pool.tile([P, bpc], F32)
    nc.gpsimd.partition_all_reduce(sr[:], s_fix[:], channels=P,
                                   reduce_op=bass_isa.ReduceOp.add)
    npr = acc_pool.tile([P, bpc], F32)
    nc.gpsimd.partition_all_reduce(npr[:], np_col[:], channels=P,
                                   reduce_op=bass_isa.ReduceOp.add)
    nnr = acc_pool.tile([P, bpc], F32)
    nc.gpsimd.partition_all_reduce(nnr[:], nn_col[:], channels=P,
                                   reduce_op=bass_isa.ReduceOp.add)

    vp = small.tile([P, bpc], F32, tag="vp")
    nc.vector.tensor_scalar(vp[:], npr[:], 0.5, None, ALU.is_gt)
    vn = small.tile([P, bpc], F32, tag="vn")
    nc.vector.tensor_scalar(vn[:], nnr[:], 0.5, None, ALU.is_gt)
    valid = small.tile([P, bpc], F32, tag="valid")
    nc.vector.tensor_mul(valid[:], vp[:], vn[:])
    nnc = small.tile([P, bpc], F32, tag="nnc")
    nc.vector.tensor_scalar_max(nnc[:], nnr[:], 1.0)
    nninv = small.tile([P, bpc], F32, tag="nninv")
    nc.vector.reciprocal(nninv[:], nnc[:])

    sv = small.tile([P, bpc], F32, tag="sv")
    nc.vector.tensor_mul(sv[:], sr[:], valid[:])
    lossv = small.tile([P, bpc], F32, tag="lossv")
    nc.vector.tensor_mul(lossv[:], sv[:], nninv[:])
    cntv = small.tile([P, bpc], F32, tag="cntv")
    nc.vector.tensor_mul(cntv[:], npr[:], valid[:])

    nc.sync.dma_start(out_ap[0:1, :], lossv[0:1, :])
    nc.sync.dma_start(out_ap[1:2, :], cntv[0:1, :])


_NC_CACHE = {}


def _build(padp, padn):
    key = (BPC, NCORES, padp, padn)
    if key in _NC_CACHE:
        return _NC_CACHE[key]
    nc = bacc.Bacc("TRN2", target_bir_lowering=False, debug=False,
                   num_devices=NCORES)
    emb = nc.dram_tensor("emb", [BPC, padp + padn, D], F32,
                         kind="ExternalInput")
    out = nc.dram_tensor("out", [2, BPC], F32, kind="ExternalOutput")
    with tile.TileContext(nc) as tc:
        with ExitStack() as ctx:
            _kernel_body(ctx, tc, emb.ap(), out.ap(), BPC, padp, padn)
    nc.compile()
    _NC_CACHE[key] = nc
    return nc


def _pack(emb, labels):
    """Per-sample label packing: pos rows, zero pad, neg rows, zero pad."""
    npos = (labels == 1).sum(axis=1)
    nneg = (labels == 0).sum(axis=1)
    padp = max(P, int(-(-npos.max() // P)) * P)
    padn = max(P, int(-(-nneg.max() // P)) * P)
    packed = np.zeros((B, padp + padn, D), np.float32)
    for b in range(B):
        pos_idx = np.nonzero(labels[b] == 1)[0]
        neg_idx = np.nonzero(labels[b] == 0)[0]
        packed[b, :len(pos_idx)] = emb[b, pos_idx]
        packed[b, padp:padp + len(neg_idx)] = emb[b, neg_idx]
    return packed, padp, padn


def kernel(embeddings: np.ndarray, labels: np.ndarray,
           _want_results=False, _trace=False) -> np.ndarray:
    emb = np.ascontiguousarray(embeddings, dtype=np.float32)
    lab = np.asarray(labels)
    assert emb.shape == (B, N, D) and lab.shape == (B, N)

    packed, padp, padn = _pack(emb, lab)
    nc = _build(padp, padn)
    in_maps = [{"emb": packed[c * BPC:(c + 1) * BPC]} for c in range(NCORES)]
    res = bass_utils.run_bass_kernel_spmd(nc, in_maps,
                                          core_ids=list(range(NCORES)),
                                          trace=_trace)
    loss_sum = 0.0
    count = 0.0
    for c in range(NCORES):
        o = res.results[c]["out"]
        loss_sum += float(o[0].sum())
        count += float(o[1].sum())
    ans = np.float32(loss_sum) / np.float32(max(count, 1.0))
    if _want_results:
        return np.float32(ans), res
    return np.float32(ans)
